# revision 1
# baseline (speedup 1.0000x reference)
"""Multi-head attention TRN2 Bass kernel for nn_MultiHeadAttention_77610059039245.

Problem: B=4, S=2048, E=1024, H=16 heads, d_head=64, causal mask,
scale = 1/sqrt(1024). f32 inputs/outputs.

Sharding (8 cores): core c = (b, g) with b = c//2 batch, g = c%2 head-group.
Each core computes heads 8g..8g+7 of batch b end-to-end (Wq/Wk/Wv column
split, Wo row split) and returns a partial output [S, E]; the host sums the
two partials per batch (the "all-reduce").

Per-core pipeline (all matmuls in fp32r — full f32 layout, 1 cycle/row):
  A) transpose x (PE, f32r 1.5cyc/row) -> xT slabs; projections
     qT/kT = [d, S] per head-pair tiles; v natural [S, d] augmented with a
     ones column (softmax denominator comes free out of the PV matmul).
  B) per head-pair, per 512-wide query strip: scores s^T = kT.T @ qT
     (two K=64 matmuls row-tiled to run concurrently), causal masking via
     additive tril tile on the diagonal blocks only (off-causal j-tiles are
     skipped entirely), exp on ACT (scale=1/32 folded in), PV matmul
     accumulates x^T[d, i] and the denominator row, then normalize with
     reciprocal + partition_broadcast + multiply.
  C) output projection out = x_norm @ Wo_part via x^T as lhsT.
"""
import numpy as np

import concourse.bass as bass
import concourse.mybir as mybir
import concourse.tile as tile
from concourse import bacc
from concourse.bass_utils import run_bass_kernel_spmd

F32 = mybir.dt.float32
F32R = mybir.dt.float32r
EXP = mybir.ActivationFunctionType.Exp

B, S, E, H = 4, 2048, 1024, 16
D = 64                    # head dim
HC = 8                    # heads per core
HP = HC // 2              # head pairs per core
GD = HC * D               # per-core projected width (512)
SCALE = 1.0 / 32.0        # 1/sqrt(QK=1024)
NEG = -1.0e5              # additive mask value (pre-scale); exp -> 0
N_CORES = 8
CH = 256                  # phase-A s-chunk
ST = S // 128             # 16 s-tiles
IT = S // 512             # 4 i-strips


def build_core_kernel(reps=1):
    nc = bacc.Bacc("TRN2", target_bir_lowering=False)

    xq = nc.dram_tensor("xqT", [E, S], F32R, kind="ExternalInput")
    xk = nc.dram_tensor("xkT", [E, S], F32R, kind="ExternalInput")
    xv = nc.dram_tensor("xvT", [E, S], F32R, kind="ExternalInput")
    wq = nc.dram_tensor("wq", [E, GD], F32R, kind="ExternalInput")
    wk = nc.dram_tensor("wk", [E, GD], F32R, kind="ExternalInput")
    wv = nc.dram_tensor("wv", [E, GD], F32R, kind="ExternalInput")
    wo = nc.dram_tensor("wo", [GD, E], F32R, kind="ExternalInput")
    tril = nc.dram_tensor("tril", [128, 128], F32, kind="ExternalInput")
    identd = nc.dram_tensor("ident", [128, 128], F32R, kind="ExternalInput")
    onesd = nc.dram_tensor("onesc", [128, 128], F32R, kind="ExternalInput")
    out = nc.dram_tensor("out", [S, E], F32, kind="ExternalOutput")

    with tile.TileContext(nc) as tc:
        with (
            tc.tile_pool(name="consts", bufs=1) as consts,
            tc.tile_pool(name="wpool", bufs=12) as wpool,
            tc.tile_pool(name="slab", bufs=3) as slabp,
            tc.tile_pool(name="qkv", bufs=1) as qkv,
            tc.tile_pool(name="pt", bufs=2) as ptp,
            tc.tile_pool(name="small", bufs=2) as small,
            tc.tile_pool(name="xto", bufs=1) as xtop,
            tc.tile_pool(name="ostage", bufs=3) as ostage,
            tc.tile_pool(name="ps", bufs=2, space="PSUM") as ps,
            tc.tile_pool(name="psx", bufs=4, space="PSUM") as psx,
        ):
            tril_t = consts.tile([128, 128], F32)
            nc.scalar.dma_start(out=tril_t, in_=tril[:, :])
            ones_t = consts.tile([128, 128], F32R)
            nc.scalar.dma_start(out=ones_t, in_=onesd[:, :])

            for _rep in range(reps):
                # persistent per-pair tensors
                qT = [qkv.tile([128, S], F32R, tag=f"qT{p}", name=f"qT{p}") for p in range(HP)]
                kT = [qkv.tile([128, S], F32R, tag=f"kT{p}", name=f"kT{p}") for p in range(HP)]
                # v augmented with ones column: [128, head, jt, 65]
                v_aug = qkv.tile([128, HC, ST, D + 1], F32R, tag="v_aug")
                nc.vector.tensor_copy(
                    v_aug[:, :, :, D:D + 1],
                    ones_t.rearrange("p (a b c) -> p a b c", a=HC, b=ST))
                xT_out = [xtop.tile([128, S], F32R, tag=f"xto{p}", name=f"xto{p}") for p in range(HP)]

                # ---------------- Phase A: projections ----------------
                def load_w(wdram):
                    tiles = []
                    for et in range(8):
                        t = wpool.tile([128, GD], F32R, tag="w")
                        eng = nc.sync if et % 2 == 0 else nc.scalar
                        eng.dma_start(out=t, in_=wdram[et * 128:(et + 1) * 128, :])
                        tiles.append(t)
                    return tiles

                def transpose_chunk(xdram, sc):
                    """DMA xT columns [sc*CH, (sc+1)*CH) into a slab [128, 8, CH]."""
                    slab = slabp.tile([128, 8, CH], F32R, tag="slab")
                    src = xdram.rearrange("(a p) s -> p a s", p=128)
                    nc.scalar.dma_start(
                        out=slab[:, 0:4], in_=src[:, 0:4, sc * CH:(sc + 1) * CH])
                    nc.sync.dma_start(
                        out=slab[:, 4:8], in_=src[:, 4:8, sc * CH:(sc + 1) * CH])
                    return slab

                def proj_qk(dest, wt, slab, sc):
                    for dp in range(HP):
                        pj = ps.tile([128, CH], F32, tag="ps", name="pj")
                        for et in range(8):
                            nc.tensor.matmul(
                                pj, wt[et][:, dp * 128:(dp + 1) * 128],
                                slab[:, et, :],
                                start=(et == 0), stop=(et == 7))
                        nc.vector.tensor_copy(dest[dp][:, sc * CH:(sc + 1) * CH], pj)

                def proj_v(wt, slab, sc):
                    for st in range(CH // 128):
                        jt = sc * (CH // 128) + st
                        pj = ps.tile([128, 512], F32, tag="ps", name="pj")
                        for et in range(8):
                            nc.tensor.matmul(
                                pj, slab[:, et, st * 128:(st + 1) * 128],
                                wt[et],
                                start=(et == 0), stop=(et == 7))
                        nc.vector.tensor_copy(
                            v_aug[:, :, jt, 0:D],
                            pj.rearrange("p (h d) -> p h d", h=HC))

                # ---------------- Phase B helpers: attention ----------------
                # software-pipelined: scores(jt+1)+mask(jt+1) are emitted before
                # exp(jt)/PV(jt), so PE runs scores while ACT runs exp; the
                # normalize for a strip is emitted after the next strip's first
                # scores so PE is not blocked at strip boundaries.
                def emit_scores(p, it, jt):
                    kdiag = jt - 4 * it
                    c0 = 128 * kdiag if kdiag > 0 else 0
                    i0 = it * 512 + c0
                    sw = ps.tile([128, 2, 512], F32, tag="ps", name="sw")
                    nc.tensor.matmul(
                        sw[:, 0, c0:], kT[p][0:64, jt * 128:(jt + 1) * 128],
                        qT[p][0:64, i0:(it + 1) * 512],
                        start=True, stop=True)
                    nc.tensor.matmul(
                        sw[:, 1, c0:], kT[p][64:128, jt * 128:(jt + 1) * 128],
                        qT[p][64:128, i0:(it + 1) * 512],
                        start=True, stop=True)
                    return sw, c0

                def emit_normalize(p, it, px1, px2):
                    for hh, px in ((0, px1), (1, px2)):
                        rrow = small.tile([1, 512], F32, tag="rrow", name="rrow")
                        nc.vector.reciprocal(rrow, px[64:65, :])
                        bc = small.tile([64, 512], F32, tag="bc", name="bc")
                        nc.gpsimd.partition_broadcast(bc, rrow)
                        nc.vector.tensor_mul(
                            xT_out[p][hh * 64:(hh + 1) * 64,
                                      it * 512:(it + 1) * 512],
                            px[0:64, :], bc)

                state = {"pending": None}  # strip awaiting normalize

                def emit_strip(p, it):
                    h1, h2 = 2 * p, 2 * p + 1
                    jmax = 4 * it + 3
                    px1 = psx.tile([128, 512], F32, tag="psx", name="px1")
                    px2 = psx.tile([128, 512], F32, tag="psx", name="px2")
                    sw_cur, c0_cur = emit_scores(p, it, 0)
                    if state["pending"] is not None:
                        emit_normalize(*state["pending"])
                        state["pending"] = None
                    for jt in range(jmax + 1):
                        if jt < jmax:
                            sw_next, c0_next = emit_scores(p, it, jt + 1)
                        pt = ptp.tile([128, 2, 512], F32R, tag="pt", name="pt")
                        c0 = c0_cur
                        nc.scalar.activation(pt[:, :, c0:], sw_cur[:, :, c0:],
                                             EXP, scale=SCALE)
                        kdiag = jt - 4 * it
                        if kdiag >= 0:
                            # zero the above-causal triangle of exp(s) (0/1 mask)
                            cs = slice(c0, c0 + 128)
                            nc.vector.tensor_mul(pt[:, 0, cs], pt[:, 0, cs], tril_t)
                            nc.vector.tensor_mul(pt[:, 1, cs], pt[:, 1, cs], tril_t)
                        nc.tensor.matmul(
                            px1[0:65, c0:], v_aug[:, h1, jt, :], pt[:, 0, c0:],
                            start=(jt == 0), stop=(jt == jmax))
                        nc.tensor.matmul(
                            px2[0:65, c0:], v_aug[:, h2, jt, :], pt[:, 1, c0:],
                            start=(jt == 0), stop=(jt == jmax))
                        if jt < jmax:
                            sw_cur, c0_cur = sw_next, c0_next
                    state["pending"] = (p, it, px1, px2)

                # ------------- fused A/B emission with software pipelining -------------
                # chunks: q/k interleaved first (strips need full qT/kT), then v
                # chunks with attention strips interleaved as their v-tiles land.
                with nc.named_scope("proj_attn"):
                    wts = {"q": load_w(wq), "k": load_w(wk), "v": load_w(wv)}
                    chunks = ([("q", xq, sc) for sc in range(S // CH)]
                              + [("k", xk, sc) for sc in range(S // CH)]
                              + [("v", xv, sc) for sc in range(S // CH)])
                    after = {}  # chunk index -> list of strips to emit after its proj
                    nvc = S // CH
                    base = 2 * (S // CH)
                    for it in range(IT):
                        # v chunks 0..2it+1 cover jt 0..4it+3
                        ci = base + 2 * it + 1
                        after[ci] = [(p, it) for p in range(HP)]
                    slab_cur = transpose_chunk(chunks[0][1], chunks[0][2])
                    for i, (nm, xd, sc) in enumerate(chunks):
                        if i + 1 < len(chunks):
                            nm2, xd2, sc2 = chunks[i + 1]
                            slab_next = transpose_chunk(xd2, sc2)
                        else:
                            slab_next = None
                        if nm == "q":
                            proj_qk(qT, wts["q"], slab_cur, sc)
                        elif nm == "k":
                            proj_qk(kT, wts["k"], slab_cur, sc)
                        else:
                            proj_v(wts["v"], slab_cur, sc)
                        slab_cur = slab_next
                        for (p, it) in after.get(i, ()):
                            emit_strip(p, it)

                    # Wo prefetch (used in phase C)
                    wot = []
                    for kt in range(4):
                        for eh in range(2):
                            t = wpool.tile([128, GD], F32R, tag="w")
                            nc.sync.dma_start(
                                out=t, in_=wo[kt * 128:(kt + 1) * 128,
                                              eh * 512:(eh + 1) * 512])
                            wot.append(t)
                    if state["pending"] is not None:
                        emit_normalize(*state["pending"])
                        state["pending"] = None

                # ---------------- Phase C: output projection ----------------
                with nc.named_scope("outproj"):
                    for st in range(ST):
                        po = ps.tile([128, 2, 512], F32, tag="ps", name="po")
                        for eh in range(2):
                            for kt in range(4):
                                nc.tensor.matmul(
                                    po[:, eh, :], xT_out[kt][:, st * 128:(st + 1) * 128],
                                    wot[kt * 2 + eh],
                                    start=(kt == 0), stop=(kt == 3))
                        ot = ostage.tile([128, 1024], F32, tag="ostage")
                        nc.vector.tensor_copy(ot, po.rearrange("p a b -> p (a b)"))
                        eng = nc.sync if st % 2 == 0 else nc.scalar
                        eng.dma_start(out=out[st * 128:(st + 1) * 128, :], in_=ot)

    nc.finalize()
    return nc


_NC = None


def _get_nc():
    global _NC
    if _NC is None:
        _NC = build_core_kernel()
    return _NC


def _tril_mask():
    # multiplicative causal mask for s^T blocks: keep j (row) <= i (col)
    r = np.arange(128)
    return np.where(r[:, None] <= r[None, :], 1.0, 0.0).astype(np.float32)


def make_in_maps(query, key, value, Wq, Wk, Wv, Wo):
    query = np.ascontiguousarray(np.asarray(query, np.float32))
    key = np.ascontiguousarray(np.asarray(key, np.float32))
    value = np.ascontiguousarray(np.asarray(value, np.float32))
    Wq = np.ascontiguousarray(np.asarray(Wq, np.float32))
    Wk = np.ascontiguousarray(np.asarray(Wk, np.float32))
    Wv = np.ascontiguousarray(np.asarray(Wv, np.float32))
    Wo = np.ascontiguousarray(np.asarray(Wo, np.float32))
    tril_m = _tril_mask()
    in_maps = []
    for c in range(N_CORES):
        b, g = c // 2, c % 2
        cols = slice(g * GD, (g + 1) * GD)
        in_maps.append({
            "xqT": np.ascontiguousarray(query[b].T),
            "xkT": np.ascontiguousarray(key[b].T),
            "xvT": np.ascontiguousarray(value[b].T),
            "wq": np.ascontiguousarray(Wq[:, cols]),
            "wk": np.ascontiguousarray(Wk[:, cols]),
            "wv": np.ascontiguousarray(Wv[:, cols]),
            "wo": np.ascontiguousarray(Wo[g * GD:(g + 1) * GD, :]),
            "tril": tril_m,
            "ident": np.eye(128, dtype=np.float32),
            "onesc": np.ones((128, 128), np.float32),
        })
    return in_maps


def kernel(query, key, value, mask, Wq, Wk, Wv, Wo, **run_kwargs):
    nc = _get_nc()
    in_maps = make_in_maps(query, key, value, Wq, Wk, Wv, Wo)
    res = run_bass_kernel_spmd(nc, in_maps, core_ids=list(range(N_CORES)),
                               **run_kwargs)
    out = np.empty((B, S, E), np.float32)
    for b in range(B):
        out[b] = res.results[2 * b]["out"] + res.results[2 * b + 1]["out"]
    if run_kwargs:
        kernel.last_result = res
    return out


if __name__ == "__main__":
    rng = np.random.default_rng(0)
    q = rng.standard_normal((B, S, E), dtype=np.float32)
    k = rng.standard_normal((B, S, E), dtype=np.float32)
    v = rng.standard_normal((B, S, E), dtype=np.float32)
    sc = 1.0 / np.sqrt(E)
    Wq = rng.standard_normal((E, E), dtype=np.float32) * sc
    Wk = rng.standard_normal((E, E), dtype=np.float32) * sc
    Wv = rng.standard_normal((E, E), dtype=np.float32) * sc
    Wo = rng.standard_normal((E, E), dtype=np.float32) * sc
    o = kernel(q, k, v, None, Wq, Wk, Wv, Wo)
    print("out", o.shape, o.dtype, float(np.abs(o).mean()))



# revision 3
# speedup vs baseline: 1.0056x; 1.0056x over previous
"""Multi-head attention TRN2 Bass kernel for nn_MultiHeadAttention_77610059039245.

Problem: B=4, S=2048, E=1024, H=16 heads, d_head=64, causal mask,
scale = 1/sqrt(1024). f32 inputs/outputs.

Sharding (8 cores): core c = (b, g) with b = c//2 batch, g = c%2 head-group.
Each core computes heads 8g..8g+7 of batch b end-to-end (Wq/Wk/Wv column
split, Wo row split) and returns a partial output [S, E]; the host sums the
two partials per batch (the "all-reduce").

All matmul operands are bf16 (host pre-converts x and W): enables FWL fast
weight loads, 1 cyc/row at any moving width, halves DMA + SBUF. PSUM
accumulation stays f32.

Per-core pipeline, chunked by 512 sequence columns (sc = strip it):
  for sc in 0..3:
    load x slabs (q,k,v chunk sc), project into qT/kT[pair] ([128,S] bf16,
    two heads stacked on partitions) and v_aug ([j, head, jt, 65] bf16 with a
    ones column so the softmax denominator falls out of the PV matmul);
    for each head-pair p: attention strip (p, it=sc): per j-tile jt<=4it+3,
    scores sT = kT.T @ qT (two row-tiled K=64 matmuls), exp on ACT
    (scale=1/32 folded), causal tril multiply on the diagonal tiles only,
    PV accumulate into PSUM [65, 512]; normalize via reciprocal +
    partition_broadcast + multiply into xT_out (bf16);
    output projection for s-tiles 4it..4it+3 (lhsT = xT_out, rhs = Wo part),
    staged PSUM->SBUF and DMA'd out. Scores are emitted two j-tiles ahead of
    PV so PE never waits on ACT.
"""
import numpy as np
import ml_dtypes

import concourse.bass as bass
import concourse.mybir as mybir
import concourse.tile as tile
from concourse import bacc
from concourse.bass_utils import run_bass_kernel_spmd

F32 = mybir.dt.float32
BF16 = mybir.dt.bfloat16
EXP = mybir.ActivationFunctionType.Exp

B, S, E, H = 4, 2048, 1024, 16
D = 64                    # head dim
HC = 8                    # heads per core
HP = HC // 2              # head pairs per core
GD = HC * D               # per-core projected width (512)
SCALE = 1.0 / 32.0        # 1/sqrt(QK=1024)
N_CORES = 8
CH = 512                  # s-chunk == strip width
ST = S // 128             # 16 s-tiles
IT = S // CH              # 4 strips


def build_core_kernel(reps=1):
    nc = bacc.Bacc("TRN2", target_bir_lowering=False)

    xq = nc.dram_tensor("xqT", [E, S], BF16, kind="ExternalInput")
    xk = nc.dram_tensor("xkT", [E, S], BF16, kind="ExternalInput")
    xv = nc.dram_tensor("xvT", [E, S], BF16, kind="ExternalInput")
    wq = nc.dram_tensor("wq", [E, GD], BF16, kind="ExternalInput")
    wk = nc.dram_tensor("wk", [E, GD], BF16, kind="ExternalInput")
    wv = nc.dram_tensor("wv", [E, GD], BF16, kind="ExternalInput")
    wo = nc.dram_tensor("wo", [GD, E], BF16, kind="ExternalInput")
    tril = nc.dram_tensor("tril", [128, 128], BF16, kind="ExternalInput")
    onesd = nc.dram_tensor("onesc", [128, 128], BF16, kind="ExternalInput")
    out = nc.dram_tensor("out", [S, E], F32, kind="ExternalOutput")

    with tile.TileContext(nc) as tc:
        with (
            tc.tile_pool(name="consts", bufs=1) as consts,
            tc.tile_pool(name="wpool", bufs=32) as wpool,
            tc.tile_pool(name="slab", bufs=6) as slabp,
            tc.tile_pool(name="qkv", bufs=1) as qkv,
            tc.tile_pool(name="pt", bufs=3) as ptp,
            tc.tile_pool(name="small", bufs=2) as small,
            tc.tile_pool(name="xto", bufs=1) as xtop,
            tc.tile_pool(name="ostage", bufs=3) as ostage,
            tc.tile_pool(name="ps", bufs=3, space="PSUM") as ps,
            tc.tile_pool(name="psx", bufs=2, space="PSUM") as psx,
        ):
            tril_t = consts.tile([128, 128], BF16)
            nc.sync.dma_start(out=tril_t, in_=tril[:, :])
            ones_t = consts.tile([128, 128], BF16)
            nc.scalar.dma_start(out=ones_t, in_=onesd[:, :])

            for _rep in range(reps):
                qT = [qkv.tile([128, S], BF16, tag=f"qT{p}", name=f"qT{p}") for p in range(HP)]
                kT = [qkv.tile([128, S], BF16, tag=f"kT{p}", name=f"kT{p}") for p in range(HP)]
                # v augmented with ones column: [128, head, jt, 65]
                v_aug = qkv.tile([128, HC, ST, D + 1], BF16, tag="v_aug")
                nc.gpsimd.tensor_copy(
                    v_aug[:, :, :, D:D + 1],
                    ones_t.rearrange("p (a b c) -> p a b c", a=HC, b=ST))
                xT_out = [xtop.tile([128, S], BF16, tag=f"xto{p}", name=f"xto{p}") for p in range(HP)]

                # ---- weights: preload everything (bf16: 32KB/partition) ----
                def load_w(wdram):
                    tiles = []
                    for et in range(8):
                        t = wpool.tile([128, GD], BF16, tag="w")
                        eng = nc.sync if et % 2 == 0 else nc.scalar
                        eng.dma_start(out=t, in_=wdram[et * 128:(et + 1) * 128, :])
                        tiles.append(t)
                    return tiles

                def transpose_chunk(xdram, sc):
                    """DMA xT columns [sc*CH, (sc+1)*CH) into a slab [128, 8, CH]."""
                    slab = slabp.tile([128, 8, CH], BF16, tag="slab")
                    src = xdram.rearrange("(a p) s -> p a s", p=128)
                    nc.scalar.dma_start(
                        out=slab[:, 0:4], in_=src[:, 0:4, sc * CH:(sc + 1) * CH])
                    nc.sync.dma_start(
                        out=slab[:, 4:8], in_=src[:, 4:8, sc * CH:(sc + 1) * CH])
                    return slab

                def proj_qk(dest, wt, slab, sc):
                    for dp in range(HP):
                        pj = ps.tile([128, 2, 512], F32, tag="ps", name="pj")
                        for et in range(8):
                            nc.tensor.matmul(
                                pj[:, 0, :],
                                wt[et][:, dp * 128:(dp + 1) * 128],
                                slab[:, et, :],
                                start=(et == 0), stop=(et == 7))
                        nc.vector.tensor_copy(
                            dest[dp][:, sc * CH:(sc + 1) * CH], pj[:, 0, :])

                def proj_v(wt, slab, sc):
                    for st in range(CH // 128):
                        jt = sc * (CH // 128) + st
                        pj = ps.tile([128, 2, 512], F32, tag="ps", name="pj")
                        for et in range(8):
                            nc.tensor.matmul(
                                pj[:, 0, :],
                                slab[:, et, st * 128:(st + 1) * 128],
                                wt[et],
                                start=(et == 0), stop=(et == 7))
                        nc.vector.tensor_copy(
                            v_aug[:, :, jt, 0:D],
                            pj[:, 0, :].rearrange("p (h d) -> p h d", h=HC))

                # ---- attention strip helpers ----
                def emit_scores(p, it, jt):
                    kdiag = jt - 4 * it
                    c0 = 128 * kdiag if kdiag > 0 else 0
                    i0 = it * 512 + c0
                    sw = ps.tile([128, 2, 512], F32, tag="ps", name="sw")
                    nc.tensor.matmul(
                        sw[:, 0, c0:], kT[p][0:64, jt * 128:(jt + 1) * 128],
                        qT[p][0:64, i0:(it + 1) * 512],
                        start=True, stop=True)
                    nc.tensor.matmul(
                        sw[:, 1, c0:], kT[p][64:128, jt * 128:(jt + 1) * 128],
                        qT[p][64:128, i0:(it + 1) * 512],
                        start=True, stop=True)
                    return sw, c0

                def emit_normalize(p, it, px1, px2):
                    for hh, px in ((0, px1), (1, px2)):
                        rrow = small.tile([1, 512], F32, tag="rrow", name="rrow")
                        nc.vector.reciprocal(rrow, px[64:65, :])
                        bc = small.tile([64, 512], F32, tag="bc", name="bc")
                        nc.gpsimd.partition_broadcast(bc, rrow)
                        nc.vector.tensor_mul(
                            xT_out[p][hh * 64:(hh + 1) * 64,
                                      it * 512:(it + 1) * 512],
                            px[0:64, :], bc)

                def emit_strip(p, it):
                    """Scores two j-tiles ahead of PV so PE never waits on ACT."""
                    h1, h2 = 2 * p, 2 * p + 1
                    jmax = 4 * it + 3
                    px1 = psx.tile([128, 512], F32, tag="psx", name="px1")
                    px2 = psx.tile([128, 512], F32, tag="psx", name="px2")
                    pend = []          # [(sw, c0), ...] scores not yet consumed
                    pend.append(emit_scores(p, it, 0))
                    if jmax >= 1:
                        pend.append(emit_scores(p, it, 1))
                    for jt in range(jmax + 1):
                        sw_cur, c0 = pend.pop(0)
                        pt = ptp.tile([128, 2, 512], BF16, tag="pt", name="pt")
                        nc.scalar.activation(pt[:, :, c0:], sw_cur[:, :, c0:],
                                             EXP, scale=SCALE)
                        kdiag = jt - 4 * it
                        if kdiag >= 0:
                            cs = slice(c0, c0 + 128)
                            nc.vector.tensor_mul(pt[:, 0, cs], pt[:, 0, cs], tril_t)
                            nc.vector.tensor_mul(pt[:, 1, cs], pt[:, 1, cs], tril_t)
                        if jt + 2 <= jmax:
                            pend.append(emit_scores(p, it, jt + 2))
                        nc.tensor.matmul(
                            px1[0:65, c0:], v_aug[:, h1, jt, :], pt[:, 0, c0:],
                            start=(jt == 0), stop=(jt == jmax))
                        nc.tensor.matmul(
                            px2[0:65, c0:], v_aug[:, h2, jt, :], pt[:, 1, c0:],
                            start=(jt == 0), stop=(jt == jmax))
                    emit_normalize(p, it, px1, px2)

                def emit_outproj_block(it, wot):
                    for st in range(4 * it, 4 * it + 4):
                        po = ps.tile([128, 2, 512], F32, tag="ps", name="po")
                        for eh in range(2):
                            for kt in range(4):
                                nc.tensor.matmul(
                                    po[:, eh, :],
                                    xT_out[kt][:, st * 128:(st + 1) * 128],
                                    wot[kt * 2 + eh],
                                    start=(kt == 0), stop=(kt == 3))
                        ot = ostage.tile([128, 1024], F32, tag="ostage")
                        nc.vector.tensor_copy(ot, po.rearrange("p a b -> p (a b)"))
                        nc.sync.dma_start(out=out[st * 128:(st + 1) * 128, :], in_=ot)

                # ---- fused schedule ----
                with nc.named_scope("mha"):
                    wts = {"q": load_w(wq), "k": load_w(wk), "v": load_w(wv)}
                    wot = []
                    for kt in range(4):
                        for eh in range(2):
                            t = wpool.tile([128, GD], BF16, tag="w")
                            eng = nc.sync if eh == 0 else nc.scalar
                            eng.dma_start(
                                out=t, in_=wo[kt * 128:(kt + 1) * 128,
                                              eh * 512:(eh + 1) * 512])
                            wot.append(t)

                    slabs_next = [transpose_chunk(xq, 0),
                                  transpose_chunk(xk, 0),
                                  transpose_chunk(xv, 0)]
                    for sc in range(IT):
                        slabs_cur = slabs_next
                        if sc + 1 < IT:
                            slabs_next = [transpose_chunk(xq, sc + 1),
                                          transpose_chunk(xk, sc + 1),
                                          transpose_chunk(xv, sc + 1)]
                        proj_qk(qT, wts["q"], slabs_cur[0], sc)
                        proj_qk(kT, wts["k"], slabs_cur[1], sc)
                        proj_v(wts["v"], slabs_cur[2], sc)
                        for p in range(HP):
                            emit_strip(p, sc)
                        emit_outproj_block(sc, wot)

    nc.finalize()
    return nc


_NC = None


def _get_nc():
    global _NC
    if _NC is None:
        _NC = build_core_kernel()
    return _NC


def _tril_mask():
    # multiplicative causal mask for s^T blocks: keep j (row) <= i (col)
    r = np.arange(128)
    return np.where(r[:, None] <= r[None, :], 1.0, 0.0).astype(ml_dtypes.bfloat16)


def make_in_maps(query, key, value, Wq, Wk, Wv, Wo):
    bf = ml_dtypes.bfloat16
    query = np.asarray(query, np.float32)
    key = np.asarray(key, np.float32)
    value = np.asarray(value, np.float32)
    Wq = np.asarray(Wq, np.float32).astype(bf)
    Wk = np.asarray(Wk, np.float32).astype(bf)
    Wv = np.asarray(Wv, np.float32).astype(bf)
    Wo = np.asarray(Wo, np.float32).astype(bf)
    tril_m = _tril_mask()
    ones_m = np.ones((128, 128), bf)
    xqT = [np.ascontiguousarray(query[b].T.astype(bf)) for b in range(B)]
    xkT = [np.ascontiguousarray(key[b].T.astype(bf)) for b in range(B)]
    xvT = [np.ascontiguousarray(value[b].T.astype(bf)) for b in range(B)]
    in_maps = []
    for c in range(N_CORES):
        b, g = c // 2, c % 2
        cols = slice(g * GD, (g + 1) * GD)
        in_maps.append({
            "xqT": xqT[b],
            "xkT": xkT[b],
            "xvT": xvT[b],
            "wq": np.ascontiguousarray(Wq[:, cols]),
            "wk": np.ascontiguousarray(Wk[:, cols]),
            "wv": np.ascontiguousarray(Wv[:, cols]),
            "wo": np.ascontiguousarray(Wo[g * GD:(g + 1) * GD, :]),
            "tril": tril_m,
            "onesc": ones_m,
        })
    return in_maps


def kernel(query, key, value, mask, Wq, Wk, Wv, Wo, **run_kwargs):
    nc = _get_nc()
    in_maps = make_in_maps(query, key, value, Wq, Wk, Wv, Wo)
    res = run_bass_kernel_spmd(nc, in_maps, core_ids=list(range(N_CORES)),
                               **run_kwargs)
    out = np.empty((B, S, E), np.float32)
    for b in range(B):
        out[b] = res.results[2 * b]["out"] + res.results[2 * b + 1]["out"]
    if run_kwargs:
        kernel.last_result = res
    return out


if __name__ == "__main__":
    rng = np.random.default_rng(0)
    q = rng.standard_normal((B, S, E), dtype=np.float32)
    k = rng.standard_normal((B, S, E), dtype=np.float32)
    v = rng.standard_normal((B, S, E), dtype=np.float32)
    sc = 1.0 / np.sqrt(E)
    Wq = rng.standard_normal((E, E), dtype=np.float32) * sc
    Wk = rng.standard_normal((E, E), dtype=np.float32) * sc
    Wv = rng.standard_normal((E, E), dtype=np.float32) * sc
    Wo = rng.standard_normal((E, E), dtype=np.float32) * sc
    o = kernel(q, k, v, None, Wq, Wk, Wv, Wo)
    print("out", o.shape, o.dtype, float(np.abs(o).mean()))
